# revision 24
# baseline (speedup 1.0000x reference)
# Linear-chain CRF log-marginals on 8 Trainium2 NeuronCores.
#
# alpha/beta recurrences run in the exp domain: the per-step
# LSE_k(alpha[k] + T[k,j]) becomes a matvec u @ exp(T) on the PE array
# (fp16 operands, fp32 PSUM accumulate), with a constant per-step prescale
# exp(-MU) folded into exp(scores) and a periodic data-dependent renorm to
# keep the fp16 carry in range.  The sequence is split into many short
# chunks run speculatively in lockstep (the chain mixes in a few steps, so
# a W-step warmup makes each chunk's carry direction exact); 32 chunk-scans
# per core share each stationary-weight load.  Cores 0-3 run the forward
# direction, cores 4-7 the backward direction.
#
# No stitching is needed: the final marginals are recovered by per-position
# normalization,  out_i = q_i - LSE_j(q_i)  with  q_i = log(uf_i * vb_i),
# where uf = Vf*exp(s-MU) from the forward scan and vb = Vb from the
# backward scan.  Every per-chunk constant, renorm factor and the partition
# function Z are constant across classes j at fixed position i, so they all
# cancel inside the LSE.  A second tiny device pass does this normalization.
import numpy as np
from contextlib import ExitStack

import concourse.bass as bass
import concourse.tile as tile
from concourse import bacc, mybir
from concourse.bass_utils import run_bass_kernel_spmd
from concourse.alu_op_type import AluOpType as ALU

F16 = mybir.dt.float16
F32 = mybir.dt.float32
F8 = mybir.dt.float8e4
AFT = mybir.ActivationFunctionType
AX = mybir.AxisListType

# ---- problem constants ----
N, C = 8192, 1024
P = 128                  # partitions
CT = C // P              # 8 contraction/output tiles of 128 classes
NCORE = 8

# ---- algorithm parameters ----
NS = 512                 # lockstep scans per core
NCH = 4 * NS             # chunks per direction (4 cores each direction)
L = N // NCH             # 8 positions per chunk
W = 1                    # warmup steps per speculative chunk
R = W + L                # rows per scan: init row + R-1 steps
MU = 7.927               # constant per-step log-prescale
B0 = 4.0                 # init offset: u_0 = exp(s_0 - B0)
RN = 32                  # renorm cadence (sigma measured at m, applied at m+1)
BCS = 64.0               # renorm rescale target (sum -> 64)
G = 1                    # steps per DMA/exp group
NSC = CT * NS            # 256 carry columns per core
RENORM_STEPS = list(range(RN, R - 1, RN))

_scan_nc = None
_epi_nc = None
TIMINGS = {}


# ---------------------------------------------------------------- builders
def build_scan_nc(timing_loop=None):
    nc = bacc.Bacc(None, target_bir_lowering=False)
    tmat = nc.declare_dram_parameter("tmat", [P, C * CT], F32, isOutput=False)
    srows = nc.declare_dram_parameter("srows", [P, R * NSC], F16, isOutput=False)
    udump = nc.declare_dram_parameter("udump", [P, R * NSC], F16, isOutput=True)
    u0dump = nc.declare_dram_parameter("u0dump", [P, CT], F16, isOutput=True)

    ngroups = (R + G - 1) // G

    with tile.TileContext(nc) as tc, ExitStack() as ctx:
        const = ctx.enter_context(tc.tile_pool(name="const", bufs=1))
        mpool = ctx.enter_context(tc.tile_pool(name="m16", bufs=1))
        tin = ctx.enter_context(tc.tile_pool(name="tin", bufs=2))
        spool = ctx.enter_context(tc.tile_pool(name="sin", bufs=2))
        espool = ctx.enter_context(tc.tile_pool(name="es", bufs=2))
        upool = ctx.enter_context(tc.tile_pool(name="u", bufs=3))
        u8pool = ctx.enter_context(tc.tile_pool(name="u8", bufs=3))
        # PSUM: group jt output blocks so each accumulator tile is <= 1 bank
        JPB = max(1, 512 // NS)          # jt blocks per bank (fp32)
        NPS = CT // JPB                  # number of accumulator pools
        PSB = 2 if NPS <= 4 else 1       # double-buffer when banks allow
        psP = [ctx.enter_context(
            tc.tile_pool(name=f"ps{i}", bufs=PSB, space="PSUM"))
            for i in range(NPS)]
        assert not RENORM_STEPS, "renorm not supported in this builder (R <= RN)"

        biasmu = const.tile([P, 1], F32)
        nc.any.memset(biasmu[:], -MU)

        # M8 = exp(tmat) fp8e4 in 3D layout [p, kt, jt*P + c]; pairs of kt
        # blocks feed DoubleRow matmuls (contraction 256 over partitions x 2)
        m8 = mpool.tile([P, CT, CT * P], F8)
        for h in range(2):
            tt = tin.tile([P, C * CT // 2], F32)
            nc.sync.dma_start(tt[:], tmat[:, h * 4096:(h + 1) * 4096])
            nc.scalar.activation(m8[:, h * 4:(h + 1) * 4, :], tt[:], AFT.Exp)

        # es groups: DMA srows slice -> exp(x - MU)
        es_tiles = [None] * ngroups

        def emit_group(g):
            lo = g * G * NSC
            hi = min(R, (g + 1) * G) * NSC
            st = spool.tile([P, G * NSC], F16)
            nc.sync.dma_start(st[:, 0:hi - lo], srows[:, lo:hi])
            et = espool.tile([P, G * NSC], F32)
            nc.scalar.activation(et[:, 0:hi - lo], st[:, 0:hi - lo], AFT.Exp,
                                 bias=biasmu[:])
            es_tiles[g] = et

        loop_cm = tc.For_i(0, timing_loop, 1) if timing_loop else ExitStack()
        with loop_cm:
            emit_group(0)

            # r = 0: u0 = es_row0 * e^(MU-B0) = exp(s_0 - B0), fp16
            u_prev = upool.tile([P, NSC], F16)
            nc.vector.tensor_scalar_mul(u_prev[:], es_tiles[0][:, 0:NSC],
                                        float(np.exp(MU - B0)))
            u8 = u8pool.tile([P, CT, NS], F8)
            nc.vector.tensor_copy(u8[:], u_prev[:])
            # only chunk 0 (scan column s=0) ever reads row 0: dump just its
            # CT strided columns instead of the full 1 MB row
            nc.sync.dma_start(u0dump[:], u_prev[:].rearrange(
                "p (k s) -> p k s", k=CT)[:, :, 0])
            for r in range(1, R):
                g, slot = divmod(r, G)
                if slot == 0 and es_tiles[g] is None:
                    emit_group(g)
                if slot == 0 and g + 1 < ngroups and es_tiles[g + 1] is None:
                    emit_group(g + 1)
                es = es_tiles[g]
                off = slot * NSC

                pst = []
                for i in range(NPS):
                    pstile = psP[i].tile([P, JPB * NS], F32, name=f"pst{i}")
                    pst.append(pstile)
                for jt in range(CT):
                    tgt = pst[jt // JPB]
                    col = (jt % JPB) * NS
                    for t in range(CT // 2):
                        nc.tensor.matmul(
                            tgt[:, col:col + NS],
                            m8[:, 2 * t:2 * t + 2, jt * P:(jt + 1) * P],
                            u8[:, 2 * t:2 * t + 2, :],
                            start=(jt % JPB == 0 and t == 0),
                            stop=(jt % JPB == JPB - 1 and t == CT // 2 - 1),
                            perf_mode=mybir.MatmulPerfMode.DoubleRow,
                        )
                # u_next = V * es (fp16) via DVE straight from PSUM.  V itself
                # is never dumped: the host reconstructs vb = ub / exp(s - MU)
                # (and the row-0 quotient is class-constant, which cancels).
                # evacuate + fp8-cast per jt block: block i's cast only gates
                # next step's t=i//2 matmuls, so early blocks overlap the tail
                # of this step's matmul stream
                u_nxt = upool.tile([P, NSC], F16)
                u8 = u8pool.tile([P, CT, NS], F8, name="u8n")
                for i in range(NPS):
                    po = i * JPB * NS
                    nc.vector.tensor_mul(u_nxt[:, po:po + JPB * NS], pst[i][:],
                                         es[:, off + po:off + po + JPB * NS])
                    nc.vector.tensor_copy(u8[:, i * JPB:(i + 1) * JPB, :],
                                          u_nxt[:, po:po + JPB * NS])
                nc.sync.dma_start(udump[:, r * NSC:(r + 1) * NSC], u_nxt[:])
                u_prev = u_nxt

    nc.finalize()
    return nc


def build_epi_nc(timing_loop=None):
    # out_i = q_i - LSE_j(q_i),  q = ln(uf * vb)
    nc = bacc.Bacc(None, target_bir_lowering=False)
    TI = N // NCORE // P     # 8 position tiles per core
    uf = nc.declare_dram_parameter("uf", [P, TI * C], F8, isOutput=False)
    vb = nc.declare_dram_parameter("vb", [P, TI * C], F8, isOutput=False)
    out = nc.declare_dram_parameter("out", [P, TI * C], F16, isOutput=True)

    with tile.TileContext(nc) as tc, ExitStack() as ctx:
        apool = ctx.enter_context(tc.tile_pool(name="ina", bufs=2))
        bpool = ctx.enter_context(tc.tile_pool(name="inb", bufs=2))
        ppool = ctx.enter_context(tc.tile_pool(name="pp", bufs=2))
        qpool = ctx.enter_context(tc.tile_pool(name="qp", bufs=2))
        opool = ctx.enter_context(tc.tile_pool(name="op", bufs=2))
        spool = ctx.enter_context(tc.tile_pool(name="sc", bufs=8))
        loop_cm = tc.For_i(0, timing_loop, 1) if timing_loop else ExitStack()
        with loop_cm:
            # whole-tensor DMAs: one descriptor set per tensor instead of
            # one per 128x1024 tile (dma_start costs ~0.6us of sequencer
            # time each, which dominated the tiled version)
            a = apool.tile([P, TI * C], F8)
            nc.sync.dma_start(a[:], uf[:])
            b = bpool.tile([P, TI * C], F8)
            nc.sync.dma_start(b[:], vb[:])
            o = opool.tile([P, TI * C], F16)
            for ti in range(TI):
                sl = slice(ti * C, (ti + 1) * C)
                # p = uf*vb and sm = sum_j p in one DVE pass; then the LSE
                # over classes is just ln(sm) -- p already IS exp(q), so no
                # Exp activation (and no Ln<->Exp table switching) is needed.
                p = ppool.tile([P, C], F32)
                sm = spool.tile([P, 1], F32)
                nc.vector.scalar_tensor_tensor(p[:], a[:, sl], 1.0, b[:, sl],
                                               ALU.mult, ALU.mult,
                                               accum_out=sm[:])
                q = qpool.tile([P, C], F16)
                nc.scalar.activation(q[:], p[:], AFT.Ln)
                lz = spool.tile([P, 1], F32)
                nc.scalar.activation(lz[:], sm[:], AFT.Ln)
                nc.vector.tensor_scalar_sub(o[:, sl], q[:], lz[:])
            nc.sync.dma_start(out[:], o[:])
    nc.finalize()
    return nc


# ---------------------------------------------------------------- host prep
def build_chunk_scores(sdir):
    """Per-direction chunk score rows [NCH, R, C] (fp32, zero-padded)."""
    SS = np.zeros((NCH, R, C), np.float32)
    for g in range(NCH):
        lo = 0 if g == 0 else g * L - W
        seg = sdir[lo:min(lo + R, N)]
        SS[g, :seg.shape[0]] = seg
    return SS


def prep_scan_inputs(scores, T):
    maps = []
    for d, (Tp, sdir) in enumerate([(T, scores), (T.T, scores[::-1])]):
        tmat = np.ascontiguousarray(
            Tp.reshape(P, CT, P, CT).transpose(0, 1, 3, 2).reshape(P, CT * CT * P),
            dtype=np.float32)
        SS = build_chunk_scores(sdir)
        for cidx in range(4):
            SSc = SS[cidx * NS:(cidx + 1) * NS]          # [NS, R, C]
            srows = np.ascontiguousarray(
                SSc.reshape(NS, R, P, CT).transpose(2, 1, 3, 0)
                .reshape(P, R * NSC)).astype(np.float16)
            maps.append({"tmat": tmat, "srows": srows})
    return maps


def gather_rows(res1, key, core_lo):
    """Gather payload rows from 4 cores' dumps -> [N, C] fp16 (seq order)."""
    out = np.empty((N, C), np.float16)
    for cidx in range(4):
        dump = res1[core_lo + cidx][key].reshape(P, R, CT, NS)
        arr = dump.transpose(3, 1, 0, 2).reshape(NS, R, C)   # [s, r, C]
        for s in range(NS):
            g = cidx * NS + s
            warm = 0 if g == 0 else W
            out[g * L:(g + 1) * L] = arr[s, warm:warm + L]
    # chunk 0's row 0 (position 0 in scan order) comes from the tiny u0 dump
    out[0] = res1[core_lo]["u0dump"].reshape(C)
    return out


def prep_epi_inputs(UF, VB):
    # Row-scale both factors so e4m3 covers each position's dynamic range
    # (per-position scales cancel in the LSE), then cast to fp8.
    import ml_dtypes
    E4 = ml_dtypes.float8_e4m3fn

    def scale8(x):
        x = x.astype(np.float32)
        mx = np.abs(x).max(axis=1, keepdims=True)
        return (x * (200.0 / mx)).astype(E4)

    UF8, VB8 = scale8(UF), scale8(VB)
    maps = []
    rows = N // NCORE
    TI = rows // P
    for k in range(NCORE):
        sl = slice(k * rows, (k + 1) * rows)
        def lay(x):
            return np.ascontiguousarray(
                x[sl].reshape(TI, P, C).transpose(1, 0, 2).reshape(P, rows * C // P))
        maps.append({"uf": lay(UF8), "vb": lay(VB8)})
    return maps


def assemble_output(res2):
    rows = N // NCORE
    out = np.empty((N, C), np.float32)
    for k in range(NCORE):
        o = res2[k]["out"].reshape(P, rows // P, C).transpose(1, 0, 2)
        out[k * rows:(k + 1) * rows] = o.reshape(rows, C).astype(np.float32)
    return out


# ---------------------------------------------------------------- emulation
def emulate_scan_core(inmap):
    import ml_dtypes
    E4 = ml_dtypes.float8_e4m3fn
    tmat = inmap["tmat"]
    M16 = np.exp(tmat.astype(np.float32)).astype(E4)
    es = np.exp(inmap["srows"].astype(np.float32) - np.float32(MU))
    ust = np.zeros((P, R * NSC), np.float16)
    u = (es[:, 0:NSC] * np.float32(np.exp(MU - B0))).astype(np.float16)
    u0d = np.ascontiguousarray(u.reshape(P, CT, NS)[:, :, 0])
    Mr = M16.astype(np.float32).reshape(P, CT, CT, P)   # [p, kt, jt, q]
    fbc = None
    for r in range(1, R):
        U = u.astype(E4).astype(np.float32).reshape(P, CT, NS)
        ps = np.einsum('pkjq,pks->qjs', Mr, U, optimize=True)
        ps = ps.reshape(P, NSC)
        un = (ps * es[:, r * NSC:(r + 1) * NSC]).astype(np.float16)
        if r - 1 in RENORM_STEPS:
            un = (un.astype(np.float32) * fbc.astype(np.float32)).astype(np.float16)
        if r in RENORM_STEPS:
            sig = un.astype(np.float32).reshape(P, CT, NS).sum(axis=(0, 1))
            f16 = (np.float32(1.0) / sig).astype(np.float16)
            fb_row = (np.float32(BCS) * f16.astype(np.float32)).astype(np.float16)
            fbc = np.broadcast_to(np.tile(fb_row, CT)[None, :], (P, NSC))
        ust[:, r * NSC:(r + 1) * NSC] = un
        u = un
    return {"udump": ust, "u0dump": u0d}


def emulate_epi_core(inmap):
    p = inmap["uf"].astype(np.float32) * inmap["vb"].astype(np.float32)
    p = np.maximum(p, 1e-30)  # guard emulated ln(0) from fp8 underflow
    q = np.log(p).astype(np.float16).astype(np.float32)
    TI = q.shape[1] // C
    sm = p.reshape(P, TI, C).sum(axis=2, keepdims=True)
    o = (q.reshape(P, TI, C) - np.log(sm)).reshape(P, TI * C)
    return {"out": o.astype(np.float16)}


# ---------------------------------------------------------------- main entry
def kernel(scores, T, simulate=False):
    import time
    global _scan_nc, _epi_nc
    scores = np.ascontiguousarray(np.asarray(scores), dtype=np.float32)
    T = np.ascontiguousarray(np.asarray(T), dtype=np.float32)

    t0 = time.time()
    in1 = prep_scan_inputs(scores, T)
    TIMINGS["prep1"] = time.time() - t0

    t0 = time.time()
    if simulate:
        res1 = [emulate_scan_core(m) for m in in1]
    else:
        if _scan_nc is None:
            tb = time.time()
            _scan_nc = build_scan_nc()
            TIMINGS["build1"] = time.time() - tb
        res1 = run_bass_kernel_spmd(_scan_nc, in1, list(range(NCORE))).results
    TIMINGS["pass1"] = time.time() - t0

    t0 = time.time()
    UF = gather_rows(res1, "udump", 0)
    UB = gather_rows(res1, "udump", 4)[::-1]
    # vb = V_bwd = u_bwd / es; at the backward exact-init row the quotient is
    # the constant e^(MU-B0), which cancels in the per-position LSE.
    VB = (UB.astype(np.float32)
          / np.exp(scores - np.float32(MU))).astype(np.float16)
    in2 = prep_epi_inputs(UF, VB)
    TIMINGS["host"] = time.time() - t0

    t0 = time.time()
    if simulate:
        res2 = [emulate_epi_core(m) for m in in2]
    else:
        if _epi_nc is None:
            tb = time.time()
            _epi_nc = build_epi_nc()
            TIMINGS["build2"] = time.time() - tb
        res2 = run_bass_kernel_spmd(_epi_nc, in2, list(range(NCORE))).results
    TIMINGS["pass2"] = time.time() - t0

    t0 = time.time()
    out = assemble_output(res2)
    TIMINGS["asm"] = time.time() - t0
    return out


# revision 25
# speedup vs baseline: 1.4456x; 1.4456x over previous
# Linear-chain CRF log-marginals on 8 Trainium2 NeuronCores.
#
# alpha/beta recurrences run in the exp domain: the per-step
# LSE_k(alpha[k] + T[k,j]) becomes a matvec u @ exp(T) on the PE array
# (fp16 operands, fp32 PSUM accumulate), with a constant per-step prescale
# exp(-MU) folded into exp(scores) and a periodic data-dependent renorm to
# keep the fp16 carry in range.  The sequence is split into many short
# chunks run speculatively in lockstep (the chain mixes in a few steps, so
# a W-step warmup makes each chunk's carry direction exact); 32 chunk-scans
# per core share each stationary-weight load.  Cores 0-3 run the forward
# direction, cores 4-7 the backward direction.
#
# No stitching is needed: the final marginals are recovered by per-position
# normalization,  out_i = q_i - LSE_j(q_i)  with  q_i = log(uf_i * vb_i),
# where uf = Vf*exp(s-MU) from the forward scan and vb = Vb from the
# backward scan.  Every per-chunk constant, renorm factor and the partition
# function Z are constant across classes j at fixed position i, so they all
# cancel inside the LSE.  A second tiny device pass does this normalization.
import numpy as np
from contextlib import ExitStack

import concourse.bass as bass
import concourse.tile as tile
from concourse import bacc, mybir
from concourse.bass_utils import run_bass_kernel_spmd
from concourse.alu_op_type import AluOpType as ALU

F16 = mybir.dt.float16
F32 = mybir.dt.float32
F8 = mybir.dt.float8e4
AFT = mybir.ActivationFunctionType
AX = mybir.AxisListType

# ---- problem constants ----
N, C = 8192, 1024
P = 128                  # partitions
CT = C // P              # 8 contraction/output tiles of 128 classes
NCORE = 8

# ---- algorithm parameters ----
NS = 512                 # lockstep scans per core
NCH = 4 * NS             # chunks per direction (4 cores each direction)
L = N // NCH             # 8 positions per chunk
W = 1                    # warmup steps per speculative chunk
R = W + L                # rows per scan: init row + R-1 steps
MU = 7.927               # constant per-step log-prescale
B0 = 4.0                 # init offset: u_0 = exp(s_0 - B0)
RN = 32                  # renorm cadence (sigma measured at m, applied at m+1)
BCS = 64.0               # renorm rescale target (sum -> 64)
G = 1                    # steps per DMA/exp group
NSC = CT * NS            # 256 carry columns per core
RENORM_STEPS = list(range(RN, R - 1, RN))

_scan_nc = None
_epi_nc = None
TIMINGS = {}


# ---------------------------------------------------------------- builders
def build_scan_nc(timing_loop=None):
    nc = bacc.Bacc(None, target_bir_lowering=False)
    tmat = nc.declare_dram_parameter("tmat", [P, C * CT], F32, isOutput=False)
    srows = nc.declare_dram_parameter("srows", [P, R * NSC], F16, isOutput=False)
    udump = nc.declare_dram_parameter("udump", [P, R * NSC], F16, isOutput=True)

    ngroups = (R + G - 1) // G

    with tile.TileContext(nc) as tc, ExitStack() as ctx:
        const = ctx.enter_context(tc.tile_pool(name="const", bufs=1))
        mpool = ctx.enter_context(tc.tile_pool(name="m16", bufs=1))
        tin = ctx.enter_context(tc.tile_pool(name="tin", bufs=2))
        spool = ctx.enter_context(tc.tile_pool(name="sin", bufs=2))
        espool = ctx.enter_context(tc.tile_pool(name="es", bufs=2))
        upool = ctx.enter_context(tc.tile_pool(name="u", bufs=3))
        u8pool = ctx.enter_context(tc.tile_pool(name="u8", bufs=3))
        # PSUM: group jt output blocks so each accumulator tile is <= 1 bank
        JPB = max(1, 512 // NS)          # jt blocks per bank (fp32)
        NPS = CT // JPB                  # number of accumulator pools
        PSB = 2 if NPS <= 4 else 1       # double-buffer when banks allow
        psP = [ctx.enter_context(
            tc.tile_pool(name=f"ps{i}", bufs=PSB, space="PSUM"))
            for i in range(NPS)]
        assert not RENORM_STEPS, "renorm not supported in this builder (R <= RN)"

        biasmu = const.tile([P, 1], F32)
        nc.any.memset(biasmu[:], -MU)

        # M8 = exp(tmat) fp8e4 in 3D layout [p, kt, jt*P + c]; pairs of kt
        # blocks feed DoubleRow matmuls (contraction 256 over partitions x 2)
        m8 = mpool.tile([P, CT, CT * P], F8)
        for h in range(2):
            tt = tin.tile([P, C * CT // 2], F32)
            nc.sync.dma_start(tt[:], tmat[:, h * 4096:(h + 1) * 4096])
            nc.scalar.activation(m8[:, h * 4:(h + 1) * 4, :], tt[:], AFT.Exp)

        # es groups: DMA srows slice -> exp(x - MU)
        es_tiles = [None] * ngroups

        def emit_group(g):
            lo = g * G * NSC
            hi = min(R, (g + 1) * G) * NSC
            st = spool.tile([P, G * NSC], F16)
            nc.sync.dma_start(st[:, 0:hi - lo], srows[:, lo:hi])
            et = espool.tile([P, G * NSC], F32)
            nc.scalar.activation(et[:, 0:hi - lo], st[:, 0:hi - lo], AFT.Exp,
                                 bias=biasmu[:])
            es_tiles[g] = et

        loop_cm = tc.For_i(0, timing_loop, 1) if timing_loop else ExitStack()
        with loop_cm:
            emit_group(0)

            # r = 0: u0 = es_row0 * e^(MU-B0) = exp(s_0 - B0), fp16
            u_prev = upool.tile([P, NSC], F16)
            nc.vector.tensor_scalar_mul(u_prev[:], es_tiles[0][:, 0:NSC],
                                        float(np.exp(MU - B0)))
            u8 = u8pool.tile([P, CT, NS], F8)
            nc.vector.tensor_copy(u8[:], u_prev[:])
            nc.sync.dma_start(udump[:, 0:NSC], u_prev[:])
            for r in range(1, R):
                g, slot = divmod(r, G)
                if slot == 0 and es_tiles[g] is None:
                    emit_group(g)
                if slot == 0 and g + 1 < ngroups and es_tiles[g + 1] is None:
                    emit_group(g + 1)
                es = es_tiles[g]
                off = slot * NSC

                pst = []
                for i in range(NPS):
                    pstile = psP[i].tile([P, JPB * NS], F32, name=f"pst{i}")
                    pst.append(pstile)
                for jt in range(CT):
                    tgt = pst[jt // JPB]
                    col = (jt % JPB) * NS
                    for t in range(CT // 2):
                        nc.tensor.matmul(
                            tgt[:, col:col + NS],
                            m8[:, 2 * t:2 * t + 2, jt * P:(jt + 1) * P],
                            u8[:, 2 * t:2 * t + 2, :],
                            start=(jt % JPB == 0 and t == 0),
                            stop=(jt % JPB == JPB - 1 and t == CT // 2 - 1),
                            perf_mode=mybir.MatmulPerfMode.DoubleRow,
                        )
                # u_next = V * es (fp16) via DVE straight from PSUM.  V itself
                # is never dumped: the host reconstructs vb = ub / exp(s - MU)
                # (and the row-0 quotient is class-constant, which cancels).
                # evacuate + fp8-cast per jt block: block i's cast only gates
                # next step's t=i//2 matmuls, so early blocks overlap the tail
                # of this step's matmul stream
                u_nxt = upool.tile([P, NSC], F16)
                u8 = u8pool.tile([P, CT, NS], F8, name="u8n")
                for i in range(NPS):
                    po = i * JPB * NS
                    nc.vector.tensor_mul(u_nxt[:, po:po + JPB * NS], pst[i][:],
                                         es[:, off + po:off + po + JPB * NS])
                    nc.vector.tensor_copy(u8[:, i * JPB:(i + 1) * JPB, :],
                                          u_nxt[:, po:po + JPB * NS])
                nc.sync.dma_start(udump[:, r * NSC:(r + 1) * NSC], u_nxt[:])
                u_prev = u_nxt

    nc.finalize()
    return nc


def build_epi_nc(timing_loop=None):
    # out_i = q_i - LSE_j(q_i),  q = ln(uf * vb)
    nc = bacc.Bacc(None, target_bir_lowering=False)
    TI = N // NCORE // P     # 8 position tiles per core
    uf = nc.declare_dram_parameter("uf", [P, TI * C], F8, isOutput=False)
    vb = nc.declare_dram_parameter("vb", [P, TI * C], F8, isOutput=False)
    out = nc.declare_dram_parameter("out", [P, TI * C], F16, isOutput=True)

    with tile.TileContext(nc) as tc, ExitStack() as ctx:
        apool = ctx.enter_context(tc.tile_pool(name="ina", bufs=2))
        bpool = ctx.enter_context(tc.tile_pool(name="inb", bufs=2))
        ppool = ctx.enter_context(tc.tile_pool(name="pp", bufs=2))
        qpool = ctx.enter_context(tc.tile_pool(name="qp", bufs=2))
        opool = ctx.enter_context(tc.tile_pool(name="op", bufs=2))
        spool = ctx.enter_context(tc.tile_pool(name="sc", bufs=8))
        loop_cm = tc.For_i(0, timing_loop, 1) if timing_loop else ExitStack()
        with loop_cm:
            # whole-tensor DMAs: one descriptor set per tensor instead of
            # one per 128x1024 tile (dma_start costs ~0.6us of sequencer
            # time each, which dominated the tiled version)
            a = apool.tile([P, TI * C], F8)
            nc.sync.dma_start(a[:], uf[:])
            b = bpool.tile([P, TI * C], F8)
            nc.sync.dma_start(b[:], vb[:])
            o = opool.tile([P, TI * C], F16)
            for ti in range(TI):
                sl = slice(ti * C, (ti + 1) * C)
                # p = uf*vb and sm = sum_j p in one DVE pass; then the LSE
                # over classes is just ln(sm) -- p already IS exp(q), so no
                # Exp activation (and no Ln<->Exp table switching) is needed.
                p = ppool.tile([P, C], F32)
                sm = spool.tile([P, 1], F32)
                nc.vector.scalar_tensor_tensor(p[:], a[:, sl], 1.0, b[:, sl],
                                               ALU.mult, ALU.mult,
                                               accum_out=sm[:])
                q = qpool.tile([P, C], F16)
                nc.scalar.activation(q[:], p[:], AFT.Ln)
                lz = spool.tile([P, 1], F32)
                nc.scalar.activation(lz[:], sm[:], AFT.Ln)
                nc.vector.tensor_scalar_sub(o[:, sl], q[:], lz[:])
            nc.sync.dma_start(out[:], o[:])
    nc.finalize()
    return nc


# ---------------------------------------------------------------- host prep
def build_chunk_scores(sdir):
    """Per-direction chunk score rows [NCH, R, C] (fp32, zero-padded)."""
    SS = np.zeros((NCH, R, C), np.float32)
    for g in range(NCH):
        lo = 0 if g == 0 else g * L - W
        seg = sdir[lo:min(lo + R, N)]
        SS[g, :seg.shape[0]] = seg
    return SS


def prep_scan_inputs(scores, T):
    maps = []
    for d, (Tp, sdir) in enumerate([(T, scores), (T.T, scores[::-1])]):
        tmat = np.ascontiguousarray(
            Tp.reshape(P, CT, P, CT).transpose(0, 1, 3, 2).reshape(P, CT * CT * P),
            dtype=np.float32)
        SS = build_chunk_scores(sdir)
        for cidx in range(4):
            SSc = SS[cidx * NS:(cidx + 1) * NS]          # [NS, R, C]
            srows = np.ascontiguousarray(
                SSc.reshape(NS, R, P, CT).transpose(2, 1, 3, 0)
                .reshape(P, R * NSC)).astype(np.float16)
            maps.append({"tmat": tmat, "srows": srows})
    return maps


def gather_rows(res1, key, core_lo):
    """Gather payload rows from 4 cores' dumps -> [N, C] fp16 (seq order)."""
    out = np.empty((N, C), np.float16)
    for cidx in range(4):
        dump = res1[core_lo + cidx][key].reshape(P, R, CT, NS)
        arr = dump.transpose(3, 1, 0, 2).reshape(NS, R, C)   # [s, r, C]
        for s in range(NS):
            g = cidx * NS + s
            warm = 0 if g == 0 else W
            out[g * L:(g + 1) * L] = arr[s, warm:warm + L]
    return out


def prep_epi_inputs(UF, VB):
    # Row-scale both factors so e4m3 covers each position's dynamic range
    # (per-position scales cancel in the LSE), then cast to fp8.
    import ml_dtypes
    E4 = ml_dtypes.float8_e4m3fn

    def scale8(x):
        x = x.astype(np.float32)
        mx = np.abs(x).max(axis=1, keepdims=True)
        return (x * (200.0 / mx)).astype(E4)

    UF8, VB8 = scale8(UF), scale8(VB)
    maps = []
    rows = N // NCORE
    TI = rows // P
    for k in range(NCORE):
        sl = slice(k * rows, (k + 1) * rows)
        def lay(x):
            return np.ascontiguousarray(
                x[sl].reshape(TI, P, C).transpose(1, 0, 2).reshape(P, rows * C // P))
        maps.append({"uf": lay(UF8), "vb": lay(VB8)})
    return maps


def assemble_output(res2):
    rows = N // NCORE
    out = np.empty((N, C), np.float32)
    for k in range(NCORE):
        o = res2[k]["out"].reshape(P, rows // P, C).transpose(1, 0, 2)
        out[k * rows:(k + 1) * rows] = o.reshape(rows, C).astype(np.float32)
    return out


# ---------------------------------------------------------------- emulation
def emulate_scan_core(inmap):
    import ml_dtypes
    E4 = ml_dtypes.float8_e4m3fn
    tmat = inmap["tmat"]
    M16 = np.exp(tmat.astype(np.float32)).astype(E4)
    es = np.exp(inmap["srows"].astype(np.float32) - np.float32(MU))
    ust = np.zeros((P, R * NSC), np.float16)
    u = (es[:, 0:NSC] * np.float32(np.exp(MU - B0))).astype(np.float16)
    ust[:, 0:NSC] = u
    Mr = M16.astype(np.float32).reshape(P, CT, CT, P)   # [p, kt, jt, q]
    fbc = None
    for r in range(1, R):
        U = u.astype(E4).astype(np.float32).reshape(P, CT, NS)
        ps = np.einsum('pkjq,pks->qjs', Mr, U, optimize=True)
        ps = ps.reshape(P, NSC)
        un = (ps * es[:, r * NSC:(r + 1) * NSC]).astype(np.float16)
        if r - 1 in RENORM_STEPS:
            un = (un.astype(np.float32) * fbc.astype(np.float32)).astype(np.float16)
        if r in RENORM_STEPS:
            sig = un.astype(np.float32).reshape(P, CT, NS).sum(axis=(0, 1))
            f16 = (np.float32(1.0) / sig).astype(np.float16)
            fb_row = (np.float32(BCS) * f16.astype(np.float32)).astype(np.float16)
            fbc = np.broadcast_to(np.tile(fb_row, CT)[None, :], (P, NSC))
        ust[:, r * NSC:(r + 1) * NSC] = un
        u = un
    return {"udump": ust}


def emulate_epi_core(inmap):
    p = inmap["uf"].astype(np.float32) * inmap["vb"].astype(np.float32)
    p = np.maximum(p, 1e-30)  # guard emulated ln(0) from fp8 underflow
    q = np.log(p).astype(np.float16).astype(np.float32)
    TI = q.shape[1] // C
    sm = p.reshape(P, TI, C).sum(axis=2, keepdims=True)
    o = (q.reshape(P, TI, C) - np.log(sm)).reshape(P, TI * C)
    return {"out": o.astype(np.float16)}


# ---------------------------------------------------------------- main entry
def kernel(scores, T, simulate=False):
    import time
    global _scan_nc, _epi_nc
    scores = np.ascontiguousarray(np.asarray(scores), dtype=np.float32)
    T = np.ascontiguousarray(np.asarray(T), dtype=np.float32)

    t0 = time.time()
    in1 = prep_scan_inputs(scores, T)
    TIMINGS["prep1"] = time.time() - t0

    t0 = time.time()
    if simulate:
        res1 = [emulate_scan_core(m) for m in in1]
    else:
        if _scan_nc is None:
            tb = time.time()
            _scan_nc = build_scan_nc()
            TIMINGS["build1"] = time.time() - tb
        res1 = run_bass_kernel_spmd(_scan_nc, in1, list(range(NCORE))).results
    TIMINGS["pass1"] = time.time() - t0

    t0 = time.time()
    UF = gather_rows(res1, "udump", 0)
    UB = gather_rows(res1, "udump", 4)[::-1]
    # vb = V_bwd = u_bwd / es; at the backward exact-init row the quotient is
    # the constant e^(MU-B0), which cancels in the per-position LSE.
    VB = (UB.astype(np.float32)
          / np.exp(scores - np.float32(MU))).astype(np.float16)
    in2 = prep_epi_inputs(UF, VB)
    TIMINGS["host"] = time.time() - t0

    t0 = time.time()
    if simulate:
        res2 = [emulate_epi_core(m) for m in in2]
    else:
        if _epi_nc is None:
            tb = time.time()
            _epi_nc = build_epi_nc()
            TIMINGS["build2"] = time.time() - tb
        res2 = run_bass_kernel_spmd(_epi_nc, in2, list(range(NCORE))).results
    TIMINGS["pass2"] = time.time() - t0

    t0 = time.time()
    out = assemble_output(res2)
    TIMINGS["asm"] = time.time() - t0
    return out
